# revision 1
# baseline (speedup 1.0000x reference)
"""DGMC (deep graph matching consensus) Trainium2 kernel.

Data-parallel over the B=8 graph-pair batch: one graph pair per NeuronCore.
Per core:
  - Dense weighted adjacency A^T[s,d] (sum of edge_attr over edges s->d) is a
    host-side format conversion of (edge_index, edge_attr); all feature
    segment-sums run on device as A @ y tensor-engine matmuls streamed fp32r.
  - psi_1 GNN in feature-major layout -> h^T [64, 512]; S_hat0 = h_s^T h_t.
  - Per consensus step: softmax pieces (row-max / exp+row-sum / reciprocal),
    r_t = E^T (r_s/Z), o_t via A_t @ (r_t W2n), and mlp(D) via the packed-relu
    trick out[s,t] = sum_r w2[r] relu(a[s,r] - c[t,r]) computed as 128 fused
    DVE/ACT ops [128,512] (4 s-rows per op, bf16) + 128 PE matmuls with
    block-diagonal Wm2 weight variants accumulating into 32-row PSUM regions.
  - bm2 is dropped (softmax is shift-invariant).
"""
from contextlib import ExitStack

import ml_dtypes  # noqa: F401
import numpy as np

# ---------------------------------------------------------------------------
# Workaround: this walrus build only accepts one sync-wait per TPB_CTRL
# instruction; split the TileContext exit-drain waits across SP nops.
import concourse.tile as tile
from concourse import mybir
from concourse.vector_clock import ScopedClock


def _patched_drain_and_barrier(self, tick_clock, wait_clock):
    nop0 = self.nc.sync.nop(nofuse=True)
    wait_clock.add_sem_waits(nop0.ins, ScopedClock({None: tick_clock.global_clock}))
    si = nop0.ins.sync_info
    if si is not None and len(si.on_wait) > 1:
        waits = list(si.on_wait)
        nop0.ins.sync_info = mybir.SyncInfo(on_wait=waits[:1], on_update=list(si.on_update))
        for i in range(1, len(waits)):
            nop = self.nc.sync.nop(nofuse=True)
            nop.ins.sync_info = mybir.SyncInfo(on_wait=waits[i:i + 1], on_update=[])
    self.nc.sync.drain()
    self.nc.all_engine_barrier()
    assert self.sems is not None
    popped = self.nc._tile_sem_poison_stack.pop()
    assert popped is self._sem_poison
    self.nc.clear_and_free_semaphores(list(self.sems.allocated().values()))
    self.nc.all_engine_barrier()


tile.TileContext._drain_and_barrier = _patched_drain_and_barrier
# ---------------------------------------------------------------------------

import concourse.bacc as bacc

F32 = mybir.dt.float32
BF16 = mybir.dt.bfloat16
FP16 = mybir.dt.float16
F32R = mybir.dt.float32r

B = 8            # graph pairs (one per core)
N = 512          # nodes per graph
E = 8192         # edges per graph
DIN = 128
DH = 64
R = 32
NB = 4
STEPS = 2
N_CORES = 8


def build_kernel(repeats=1):
    nc = bacc.Bacc("TRN2", dynamic_dma_scratch_size=32768)

    xsT = nc.declare_dram_parameter("xsT", [DIN, N], F32, isOutput=False)
    xtT = nc.declare_dram_parameter("xtT", [DIN, N], F32, isOutput=False)
    rs_rows = nc.declare_dram_parameter("rs_rows", [128, STEPS, NB, R], F32, isOutput=False)
    rsT = nc.declare_dram_parameter("rsT", [STEPS, R, N], F32R, isOutput=False)
    A_in = nc.declare_dram_parameter("A_rows", [2, 128, NB, N], F32, isOutput=False)
    W1r = nc.declare_dram_parameter("W1r", [DIN, DH], F32, isOutput=False)
    W1n = nc.declare_dram_parameter("W1n", [DIN, DH], F32, isOutput=False)
    b1 = nc.declare_dram_parameter("b1", [DH, 1], F32, isOutput=False)
    W2r = nc.declare_dram_parameter("W2r", [R, R], F32R, isOutput=False)
    W2n = nc.declare_dram_parameter("W2n", [R, R], F32R, isOutput=False)
    b2 = nc.declare_dram_parameter("b2", [R, 1], F32, isOutput=False)
    Wm1 = nc.declare_dram_parameter("Wm1", [R, R], F32R, isOutput=False)
    bm1 = nc.declare_dram_parameter("bm1", [R, 1], F32, isOutput=False)
    W2p = nc.declare_dram_parameter("W2p", [128, 8, 128], FP16, isOutput=False)
    ident = nc.declare_dram_parameter("ident", [128, 128], F32R, isOutput=False)
    S0 = nc.declare_dram_parameter("S0", [N, N], F32, isOutput=True)
    SL = nc.declare_dram_parameter("SL", [N, N], F32, isOutput=True)

    with tile.TileContext(nc) as tc, ExitStack() as ctx:
        const = ctx.enter_context(tc.tile_pool(name="const", bufs=1))
        work = ctx.enter_context(tc.tile_pool(name="work", bufs=2))
        r4p = ctx.enter_context(tc.tile_pool(name="r4p", bufs=6))
        pbig = ctx.enter_context(tc.tile_pool(name="pbig", bufs=4, space="PSUM"))
        psmall = ctx.enter_context(tc.tile_pool(name="psmall", bufs=4, space="PSUM"))

        # load order: small compute-critical params first, big/late tensors last
        w1n = const.tile([DIN, DH], F32); nc.sync.dma_start(w1n[:], W1n[:])
        w1r = const.tile([DIN, DH], F32); nc.sync.dma_start(w1r[:], W1r[:])
        b1c = const.tile([DH, 1], F32); nc.sync.dma_start(b1c[:], b1[:])
        xs = const.tile([DIN, N], F32); nc.sync.dma_start(xs[:], xsT[:])
        xt = const.tile([DIN, N], F32); nc.sync.dma_start(xt[:], xtT[:])
        # warm the ACT function table immediately (costs ~2.7us once)
        actwarm = const.tile([DH, 1], F32)
        nc.scalar.activation(actwarm[:], b1c[:], mybir.ActivationFunctionType.Relu)
        w2r = const.tile([R, R], F32R); nc.sync.dma_start(w2r[:], W2r[:])
        w2n = const.tile([R, R], F32R); nc.sync.dma_start(w2n[:], W2n[:])
        b2c = const.tile([R, 1], F32); nc.sync.dma_start(b2c[:], b2[:])
        wm1 = const.tile([R, R], F32R); nc.sync.dma_start(wm1[:], Wm1[:])
        bm1c = const.tile([R, 1], F32); nc.sync.dma_start(bm1c[:], bm1[:])
        rst0 = const.tile([R, N], F32R); nc.sync.dma_start(rst0[:], rsT[0])
        rst1 = const.tile([R, N], F32R); nc.sync.dma_start(rst1[:], rsT[1])
        rsr = const.tile([128, STEPS, NB, R], F32)
        nc.sync.dma_start(rsr[:], rs_rows[:])
        rst_k = [rst0, rst1]
        rsr_k = [rsr[:, 0], rsr[:, 1]]

        A_sb, A_sbr = [], []
        for d in range(2):
            asb = const.tile([128, NB, N], F32, name=f"A_sb{d}")
            for sc_ in range(NB):
                nc.sync.dma_start(asb[:, sc_, :], A_in[d, :, sc_, :])
            A_sb.append(asb)
            asbr = const.tile([128, NB, N], F32R, name=f"A_sbr{d}")
            nc.scalar.copy(asbr[:], asb[:])
            A_sbr.append(asbr)
        A_s, A_t = A_sb
        A_sr, A_tr = A_sbr
        idn = const.tile([128, 128], F32R); nc.sync.dma_start(idn[:], ident[:])
        w2p = const.tile([128, 8, 128], FP16)
        nc.sync.dma_start(w2p[:], W2p[:])

        def add_aggT(ps, A, y_rows, cols, stop=True):
            """ps[f, d] += sum_s y[s, f] A^T[s, d] (agg arrives transposed)."""
            for sc in range(NB):
                nc.tensor.matmul(
                    ps[:], lhsT=y_rows[:, sc, cols], rhs=A[:, sc, :],
                    start=False, stop=(stop and sc == NB - 1),
                    skip_group_check=True)
            return ps

        def rows_from_psums(psums, Fdim, name, dt=F32R):
            t = work.tile([128, NB, Fdim], dt, tag=name, name=name)
            for db in range(NB):
                nc.vector.tensor_copy(t[:, db, :], psums[db][:])
            return t

        def body():
            # -------- psi_1 --------
            def psi1(xT, A, name):
                yps = []
                for nb_ in range(NB):
                    ps = psmall.tile([128, DH], F32, tag="small", name=f"y{name}{nb_}")
                    nc.tensor.matmul(ps[:], lhsT=xT[:, nb_ * 128:(nb_ + 1) * 128],
                                     rhs=w1n[:], start=True, stop=True)
                    yps.append(ps)
                y_rows = rows_from_psums(yps, DH, f"y{name}_rows", dt=F32)
                hps = psmall.tile([DH, N], F32, tag="small", name=f"h{name}ps")
                nc.tensor.matmul(hps[:], lhsT=w1r[:], rhs=xT[:], start=True, stop=False,
                                 skip_group_check=True)
                add_aggT(hps, A, y_rows, slice(0, DH))
                h = work.tile([DH, N], F32, tag=f"h{name}", name=f"h{name}")
                nc.scalar.activation(h[:], hps[:], mybir.ActivationFunctionType.Relu,
                                     bias=b1c[:, :1])
                return h

            h_s = psi1(xs, A_s, "s")
            h_t = psi1(xt, A_t, "t")

            # -------- S_hat0 --------
            S_hat = const.tile([128, NB, N], F32, name="S_hat")
            for sb in range(NB):
                ps = pbig.tile([128, N], F32, tag="big", name=f"sh0{sb}")
                nc.tensor.matmul(ps[:], lhsT=h_s[:, sb * 128:(sb + 1) * 128],
                                 rhs=h_t[:], start=True, stop=True)
                nc.vector.tensor_copy(S_hat[:, sb, :], ps[:])

            # -------- psi_2 graph-s (both steps, upfront) --------
            y0ps = []
            for k in range(STEPS):
                for nb_ in range(NB):
                    ps = psmall.tile([128, R], F32, tag="small", name=f"y0_{k}{nb_}")
                    nc.tensor.matmul(ps[:], lhsT=rst_k[k][:, nb_ * 128:(nb_ + 1) * 128],
                                     rhs=w2n[:], start=True, stop=True)
                    y0ps.append(ps)
            y0packed = work.tile([128, NB, 2 * R], F32R, tag="y0packed", name="y0packed")
            for k in range(STEPS):
                for nb_ in range(NB):
                    nc.vector.tensor_copy(y0packed[:, nb_, k * R:(k + 1) * R],
                                          y0ps[k * NB + nb_][:])
            A_packed = []
            for k in range(STEPS):
                osps = psmall.tile([R, N], F32, tag="small", name=f"osps{k}")
                nc.tensor.matmul(osps[:], lhsT=w2r[:], rhs=rst_k[k][:],
                                 start=True, stop=False, skip_group_check=True)
                add_aggT(osps, A_sr, y0packed, slice(k * R, (k + 1) * R))
                o_sT = work.tile([R, N], F32R, tag="o_sT", name=f"o_sT{k}")
                nc.scalar.activation(o_sT[:], osps[:],
                                     mybir.ActivationFunctionType.Relu, bias=b2c[:, :1])
                aps = psmall.tile([R, N], F32, tag="small", name=f"aT{k}ps")
                nc.tensor.matmul(aps[:], lhsT=wm1[:], rhs=o_sT[:],
                                 start=True, stop=True)
                aT = work.tile([R, N], F32, tag=f"aT{k}", name=f"aT{k}")
                nc.scalar.activation(aT[:], aps[:], mybir.ActivationFunctionType.Identity,
                                     bias=bm1c[:, :1])
                ap_t = const.tile([128, 128], F32, name=f"A_packed{k}")
                for j in range(4):
                    nc.vector.tensor_copy(
                        ap_t[R * j:R * (j + 1), :],
                        aT[:].rearrange("r (m j) -> r j m", j=4)[:, j, :])
                A_packed.append(ap_t)

            # -------- consensus steps --------
            mx = work.tile([128, NB], F32, tag="mx", name="mx")
            Z = work.tile([128, NB], F32, tag="Z", name="Z")
            rz = work.tile([128, NB], F32, tag="rz", name="rz")
            for k in range(STEPS):
                S_exp = work.tile([128, NB, N], F32, tag="S_exp", name=f"S_exp{k}")
                for sb in range(NB):
                    nc.vector.tensor_reduce(mx[:, sb:sb + 1], S_hat[:, sb, :],
                                            axis=mybir.AxisListType.X,
                                            op=mybir.AluOpType.max, negate=True)
                    nc.scalar.activation(S_exp[:, sb, :], S_hat[:, sb, :],
                                         mybir.ActivationFunctionType.Exp,
                                         bias=mx[:, sb:sb + 1],
                                         accum_out=Z[:, sb:sb + 1])
                nc.vector.reciprocal(rz[:], Z[:])
                if k == 0:
                    S0_sb = work.tile([128, NB, N], F32, tag="S0_sb", name="S0_sb")
                    for sb in range(NB):
                        nc.vector.tensor_scalar(
                            out=S0_sb[:, sb, :], in0=S_exp[:, sb, :],
                            scalar1=rz[:, sb:sb + 1], scalar2=None,
                            op0=mybir.AluOpType.mult)
                    nc.sync.dma_start(S0.rearrange("(a b) t -> b a t", b=128), S0_sb[:])
                rsp = work.tile([128, NB, R], F32, tag="rsp", name=f"rsp{k}")
                for sb in range(NB):
                    nc.vector.tensor_scalar(
                        out=rsp[:, sb, :], in0=rsr_k[k][:, sb, :],
                        scalar1=rz[:, sb:sb + 1], scalar2=None,
                        op0=mybir.AluOpType.mult)
                rtps = []
                for tb in range(NB):
                    ps = psmall.tile([128, R], F32, tag="small", name=f"rt{k}{tb}")
                    for sb in range(NB):
                        nc.tensor.matmul(ps[:], lhsT=S_exp[:, sb, tb * 128:(tb + 1) * 128],
                                         rhs=rsp[:, sb, :], start=(sb == 0),
                                         stop=(sb == NB - 1))
                    rtps.append(ps)
                rt_rows = rows_from_psums(rtps, R, "rt_rows")
                rtT_ps = psmall.tile([R, N], F32R, tag="small", name=f"rtTps{k}")
                for db in range(NB):
                    nc.tensor.matmul(
                        rtT_ps[:, db * 128:(db + 1) * 128], lhsT=rt_rows[:, db, :R],
                        rhs=idn[:], is_transpose=True, start=True, stop=True,
                        skip_group_check=True)
                rtT = work.tile([R, N], F32R, tag="rtT", name=f"rtT{k}")
                nc.scalar.copy(rtT[:], rtT_ps[:])
                y1ps = []
                for nb_ in range(NB):
                    ps = psmall.tile([128, R], F32, tag="small", name=f"y1_{k}{nb_}")
                    nc.tensor.matmul(ps[:], lhsT=rtT[:, nb_ * 128:(nb_ + 1) * 128],
                                     rhs=w2n[:], start=True, stop=True)
                    y1ps.append(ps)
                y1_rows = rows_from_psums(y1ps, R, "y1_rows")
                otps = psmall.tile([R, N], F32, tag="small", name=f"otT{k}")
                nc.tensor.matmul(otps[:], lhsT=w2r[:], rhs=rtT[:], start=True, stop=False,
                                 skip_group_check=True)
                add_aggT(otps, A_tr, y1_rows, slice(0, R))
                o_tT = work.tile([R, N], F32R, tag="o_tT", name=f"o_tT{k}")
                nc.scalar.activation(o_tT[:], otps[:], mybir.ActivationFunctionType.Relu,
                                     bias=b2c[:, :1])
                cps = psmall.tile([R, N], F32, tag="small", name=f"cT{k}")
                nc.tensor.matmul(cps[:], lhsT=wm1[:], rhs=o_tT[:], start=True, stop=True)
                cT4neg = work.tile([128, N], FP16, tag="cT4neg", name=f"cT4neg{k}")
                for j in range(4):
                    nc.scalar.mul(cT4neg[R * j:R * (j + 1), :], cps[:], -1.0)
                mlp_ps = [pbig.tile([128, N], F32, tag="big", name=f"mlp{k}{b_}")
                          for b_ in range(NB)]
                for m in range(128):
                    r4 = r4p.tile([128, N], FP16, tag="r4")
                    if m % 4 != 3:
                        nc.vector.tensor_scalar(
                            out=r4[:], in0=cT4neg[:],
                            scalar1=A_packed[k][:, m:m + 1], scalar2=0.0,
                            op0=mybir.AluOpType.add, op1=mybir.AluOpType.max)
                    else:
                        nc.scalar.activation(r4[:], cT4neg[:],
                                             mybir.ActivationFunctionType.Relu,
                                             bias=A_packed[k][:, m:m + 1])
                    blk, grp, v = m // 32, (m // 8) % 4, m % 8
                    if grp < 2:
                        nc.tensor.matmul(mlp_ps[blk][32 * grp:32 * (grp + 1), :],
                                         lhsT=w2p[:, v, 0:32], rhs=r4[:],
                                         start=(v == 0), stop=(v == 7))
                    elif grp == 2:
                        nc.tensor.matmul(mlp_ps[blk][64:128, :],
                                         lhsT=w2p[:, v, 0:64], rhs=r4[:],
                                         start=(v == 0), stop=False)
                    else:
                        nc.tensor.matmul(mlp_ps[blk][64:128, :],
                                         lhsT=w2p[:, v, 64:128], rhs=r4[:],
                                         start=False, stop=(v == 7))
                for sb in range(NB):
                    nc.vector.tensor_tensor(out=S_hat[:, sb, :], in0=S_hat[:, sb, :],
                                            in1=mlp_ps[sb][:], op=mybir.AluOpType.add)

            # -------- final softmax -> SL --------
            SL_sb = work.tile([128, NB, N], F32, tag="S0_sb", name="SL_sb")
            for sb in range(NB):
                nc.vector.tensor_reduce(mx[:, sb:sb + 1], S_hat[:, sb, :],
                                        axis=mybir.AxisListType.X,
                                        op=mybir.AluOpType.max, negate=True)
                nc.scalar.activation(SL_sb[:, sb, :], S_hat[:, sb, :],
                                     mybir.ActivationFunctionType.Exp,
                                     bias=mx[:, sb:sb + 1], accum_out=Z[:, sb:sb + 1])
            nc.vector.reciprocal(rz[:], Z[:])
            SLv = SL.rearrange("(a b) t -> a b t", b=128)
            for sb in range(NB):
                nc.vector.tensor_scalar(out=SL_sb[:, sb, :], in0=SL_sb[:, sb, :],
                                        scalar1=rz[:, sb:sb + 1], scalar2=None,
                                        op0=mybir.AluOpType.mult)
                nc.sync.dma_start(SLv[sb], SL_sb[:, sb, :])

        if repeats == 1:
            body()
        else:
            with tc.For_i(0, repeats, 1):
                body()

    nc.compile()
    return nc


def prep_core_inputs(g, inp):
    n0 = g * N
    xs = np.asarray(inp["x_s"][n0:n0 + N], np.float32)
    xt = np.asarray(inp["x_t"][n0:n0 + N], np.float32)
    es = np.asarray(inp["edge_index_s"][:, g * E:(g + 1) * E]).astype(np.int64) - n0
    et = np.asarray(inp["edge_index_t"][:, g * E:(g + 1) * E]).astype(np.int64) - n0
    eas = np.asarray(inp["edge_attr_s"][g * E:(g + 1) * E, 0], np.float64)
    eat = np.asarray(inp["edge_attr_t"][g * E:(g + 1) * E, 0], np.float64)
    rs = np.asarray(inp["r_s_all"][:, g], np.float32)

    def a_build(edges, ea):
        at = np.bincount(edges[0] * N + edges[1], weights=ea, minlength=N * N)
        at = at.astype(np.float32).reshape(NB, 128, N).transpose(1, 0, 2)
        return np.ascontiguousarray(at)

    w2 = np.asarray(inp["Wm2"], np.float32)[:, 0]
    w2p = np.zeros((128, 8, 128), np.float32)
    for v in range(8):
        for j in range(4):
            w2p[R * j:R * (j + 1), v, 4 * v + j] = w2
            w2p[R * j:R * (j + 1), v, 96 + 4 * v + j] = w2
    return {
        "xsT": np.ascontiguousarray(xs.T),
        "xtT": np.ascontiguousarray(xt.T),
        "rs_rows": np.ascontiguousarray(rs.reshape(STEPS, NB, 128, R).transpose(2, 0, 1, 3)),
        "rsT": np.ascontiguousarray(rs.transpose(0, 2, 1)),
        "A_rows": np.stack([a_build(es, eas), a_build(et, eat)]),
        "W1r": np.asarray(inp["W1_root"], np.float32),
        "W1n": np.asarray(inp["W1_nbr"], np.float32),
        "b1": np.asarray(inp["b1"], np.float32).reshape(DH, 1),
        "W2r": np.asarray(inp["W2_root"], np.float32),
        "W2n": np.asarray(inp["W2_nbr"], np.float32),
        "b2": np.asarray(inp["b2"], np.float32).reshape(R, 1),
        "Wm1": np.asarray(inp["Wm1"], np.float32),
        "bm1": np.asarray(inp["bm1"], np.float32).reshape(R, 1),
        "W2p": w2p.astype(np.float16),
        "ident": np.eye(128, dtype=np.float32),
    }


_NC_CACHE = {}


def _get_nc(repeats=1):
    if repeats not in _NC_CACHE:
        _NC_CACHE[repeats] = build_kernel(repeats)
    return _NC_CACHE[repeats]


def kernel(**inputs):
    from concourse.bass_utils import run_bass_kernel_spmd
    nc = _get_nc(1)
    in_maps = [prep_core_inputs(g, inputs) for g in range(B)]
    res = run_bass_kernel_spmd(nc, in_maps, core_ids=list(range(N_CORES)))
    S0 = np.stack([res.results[g]["S0"] for g in range(B)])
    SL = np.stack([res.results[g]["SL"] for g in range(B)])
    return S0, SL



# revision 2
# speedup vs baseline: 8.3013x; 8.3013x over previous
"""DGMC (deep graph matching consensus) Trainium2 kernel, v2.

Data-parallel over the B=8 graph-pair batch: one graph pair per NeuronCore.
v2 restructuring vs v1 (all numerics still within rel 2e-2):
  - everything on the PE runs f32r/fp16 (f32 matmuls were 4 cycles/col).
  - rtT computed directly as 4 accumulating matmuls
    rtT[r,t] = sum_s rsp[s,r]*S_exp[s,t] (replaces 16 mm + 4 copies + 4
    transposes + 2 copies); for step 0 the normalized S0 block feeds it.
  - cT4neg / aT-replication via 4x-tiled Wm1 lhsT (one matmul instead of
    matmul + 4 scalar muls).
  - softmax pieces read matmul PSUM directly; per-block softmax of step
    k+1 (and the final SL softmax + DMA-out) are interleaved into step
    k's 128-iteration mlp loop right after each block's PSUM retires.
  - y-psums gathered in one bank -> single [128, N*F] copy each.
  - r4 relu ops split 96/32 across DVE/ACT (gpsimd measured ~8us/op on
    HW -- never use it for the inner loop), r4 pool 12 bufs deep.
Measured marginal cost per body on HW: ~58us/iter (v1 baseline: ~188us).
"""
from contextlib import ExitStack

import ml_dtypes  # noqa: F401
import numpy as np

# ---------------------------------------------------------------------------
# Workaround: this walrus build only accepts one sync-wait per TPB_CTRL
# instruction; split the TileContext exit-drain waits across SP nops.
import concourse.tile as tile
from concourse import mybir
from concourse.vector_clock import ScopedClock


def _patched_drain_and_barrier(self, tick_clock, wait_clock):
    nop0 = self.nc.sync.nop(nofuse=True)
    wait_clock.add_sem_waits(nop0.ins, ScopedClock({None: tick_clock.global_clock}))
    si = nop0.ins.sync_info
    if si is not None and len(si.on_wait) > 1:
        waits = list(si.on_wait)
        nop0.ins.sync_info = mybir.SyncInfo(on_wait=waits[:1], on_update=list(si.on_update))
        for i in range(1, len(waits)):
            nop = self.nc.sync.nop(nofuse=True)
            nop.ins.sync_info = mybir.SyncInfo(on_wait=waits[i:i + 1], on_update=[])
    self.nc.sync.drain()
    self.nc.all_engine_barrier()
    assert self.sems is not None
    popped = self.nc._tile_sem_poison_stack.pop()
    assert popped is self._sem_poison
    self.nc.clear_and_free_semaphores(list(self.sems.allocated().values()))
    self.nc.all_engine_barrier()


tile.TileContext._drain_and_barrier = _patched_drain_and_barrier
# ---------------------------------------------------------------------------

import concourse.bacc as bacc

F32 = mybir.dt.float32
BF16 = mybir.dt.bfloat16
FP16 = mybir.dt.float16
F32R = mybir.dt.float32r

B = 8            # graph pairs (one per core)
N = 512          # nodes per graph
E = 8192         # edges per graph
DIN = 128
DH = 64
R = 32
NB = 4
STEPS = 2
N_CORES = 8

# engine for each r4 op, by v = m % 8
R4_ENGINE = {0: "dve", 1: "dve", 2: "dve", 3: "dve", 4: "dve",
             5: "dve", 6: "act", 7: "act"}


def build_kernel(repeats=1, unroll=1, r4_engine=None, r4bufs=12):
    global R4_ENGINE
    if r4_engine is not None:
        R4_ENGINE = r4_engine
    nc = bacc.Bacc("TRN2", dynamic_dma_scratch_size=32768)

    xsT = nc.declare_dram_parameter("xsT", [DIN, N], F32, isOutput=False)
    xtT = nc.declare_dram_parameter("xtT", [DIN, N], F32, isOutput=False)
    rs_rows = nc.declare_dram_parameter("rs_rows", [128, STEPS, NB, R], F32R, isOutput=False)
    rsT = nc.declare_dram_parameter("rsT", [STEPS, R, N], F32R, isOutput=False)
    A_in = nc.declare_dram_parameter("A_rows", [2, 128, NB, N], F32, isOutput=False)
    W1r = nc.declare_dram_parameter("W1r", [DIN, DH], F32, isOutput=False)
    W1n = nc.declare_dram_parameter("W1n", [DIN, DH], F32, isOutput=False)
    b1 = nc.declare_dram_parameter("b1", [DH, 1], F32, isOutput=False)
    W2r = nc.declare_dram_parameter("W2r", [R, R], F32R, isOutput=False)
    W2n = nc.declare_dram_parameter("W2n", [R, R], F32R, isOutput=False)
    b2 = nc.declare_dram_parameter("b2", [R, 1], F32, isOutput=False)
    Wm1p4 = nc.declare_dram_parameter("Wm1p4", [R, 128], F32R, isOutput=False)
    Wm1n4 = nc.declare_dram_parameter("Wm1n4", [R, 128], F32R, isOutput=False)
    bm1 = nc.declare_dram_parameter("bm1", [R, 1], F32, isOutput=False)
    W2p = nc.declare_dram_parameter("W2p", [128, 8, 128], FP16, isOutput=False)
    S0 = nc.declare_dram_parameter("S0", [N, N], F32R, isOutput=True)
    SL = nc.declare_dram_parameter("SL", [N, N], F32, isOutput=True)

    with tile.TileContext(nc) as tc, ExitStack() as ctx:
        const = ctx.enter_context(tc.tile_pool(name="const", bufs=1))
        work = ctx.enter_context(tc.tile_pool(name="work", bufs=2))
        r4p = ctx.enter_context(tc.tile_pool(name="r4p", bufs=r4bufs))
        pbig = ctx.enter_context(tc.tile_pool(name="pbig", bufs=4, space="PSUM"))
        psmall = ctx.enter_context(tc.tile_pool(name="psmall", bufs=4, space="PSUM"))

        # load order: compute-critical first
        w1n = const.tile([DIN, DH], F32); nc.sync.dma_start(w1n[:], W1n[:])
        w1r = const.tile([DIN, DH], F32); nc.sync.dma_start(w1r[:], W1r[:])
        b1c = const.tile([DH, 1], F32); nc.sync.dma_start(b1c[:], b1[:])
        xs = const.tile([DIN, N], F32); nc.sync.dma_start(xs[:], xsT[:])
        xt = const.tile([DIN, N], F32); nc.sync.dma_start(xt[:], xtT[:])
        # warm the ACT function table immediately (costs ~2.7us once)
        actwarm = const.tile([DH, 1], F32)
        nc.scalar.activation(actwarm[:], b1c[:], mybir.ActivationFunctionType.Relu)
        A_sb, A_sbr = [], []
        for d in range(2):
            asb = const.tile([128, NB, N], F32, name=f"A_sb{d}")
            nc.sync.dma_start(asb[:], A_in[d])
            A_sb.append(asb)
            asbr = const.tile([128, NB, N], F32R, name=f"A_sbr{d}")
            nc.scalar.copy(asbr[:], asb[:])
            A_sbr.append(asbr)
        A_s, A_t = A_sb
        A_sr, A_tr = A_sbr
        rst0 = const.tile([R, N], F32R); nc.sync.dma_start(rst0[:], rsT[0])
        rst1 = const.tile([R, N], F32R); nc.sync.dma_start(rst1[:], rsT[1])
        rsr = const.tile([128, STEPS, NB, R], F32R)
        nc.sync.dma_start(rsr[:], rs_rows[:])
        w2r = const.tile([R, R], F32R); nc.sync.dma_start(w2r[:], W2r[:])
        w2n = const.tile([R, R], F32R); nc.sync.dma_start(w2n[:], W2n[:])
        b2c = const.tile([R, 1], F32); nc.sync.dma_start(b2c[:], b2[:])
        wm1p4 = const.tile([R, 128], F32R); nc.sync.dma_start(wm1p4[:], Wm1p4[:])
        wm1n4 = const.tile([R, 128], F32R); nc.sync.dma_start(wm1n4[:], Wm1n4[:])
        bm1c = const.tile([R, 1], F32); nc.sync.dma_start(bm1c[:], bm1[:])
        w2p = const.tile([128, 8, 128], FP16)
        nc.sync.dma_start(w2p[:], W2p[:])
        rst_k = [rst0, rst1]

        def add_aggT(ps, A, y_rows, stop=True):
            """ps[f, d] += sum_s y[s, f] A^T[s, d] (agg arrives transposed)."""
            for sc in range(NB):
                nc.tensor.matmul(
                    ps[:], lhsT=y_rows[:, sc, :], rhs=A[:, sc, :],
                    start=False, stop=(stop and sc == NB - 1),
                    skip_group_check=True)
            return ps

        def body(it=0):
            # -------- psi_1 --------
            def psi1(xT, A, name):
                yps = psmall.tile([128, NB, DH], F32, tag="small", name=f"y{name}")
                for nb_ in range(NB):
                    nc.tensor.matmul(yps[:, nb_, :], lhsT=xT[:, nb_ * 128:(nb_ + 1) * 128],
                                     rhs=w1n[:], start=True, stop=True,
                                     skip_group_check=True)
                y_rows = work.tile([128, NB, DH], F32, tag=f"y{name}_rows",
                                   name=f"y{name}_rows")
                nc.vector.tensor_copy(y_rows[:], yps[:])
                hps = psmall.tile([DH, N], F32, tag="small", name=f"h{name}ps")
                nc.tensor.matmul(hps[:], lhsT=w1r[:], rhs=xT[:], start=True, stop=False,
                                 skip_group_check=True)
                add_aggT(hps, A, y_rows)
                h = work.tile([DH, N], F32, tag=f"h{name}", name=f"h{name}")
                nc.scalar.activation(h[:], hps[:], mybir.ActivationFunctionType.Relu,
                                     bias=b1c[:, :1])
                return h

            h_s = psi1(xs, A_s, "s")
            h_t = psi1(xt, A_t, "t")

            # -------- S_hat0 + step-0 softmax (from PSUM) --------
            S_hat = work.tile([128, NB, N], F32, tag="S_hat", name="S_hat")
            mx = work.tile([128, NB, 3], F32, tag="mx", name="mx")
            Z = work.tile([128, NB, 3], F32, tag="Z", name="Z")
            rz = work.tile([128, NB, 3], F32, tag="rz", name="rz")
            S_exp0 = work.tile([128, NB, N], F32R, tag="S_exp0", name="S_exp0")
            for sb in range(NB):
                ps = pbig.tile([128, N], F32, tag="big", name=f"sh0{sb}")
                nc.tensor.matmul(ps[:], lhsT=h_s[:, sb * 128:(sb + 1) * 128],
                                 rhs=h_t[:], start=True, stop=True)
                nc.vector.tensor_copy(S_hat[:, sb, :], ps[:])
                nc.vector.tensor_reduce(mx[:, sb, 0:1], ps[:],
                                        axis=mybir.AxisListType.X,
                                        op=mybir.AluOpType.max, negate=True)
                nc.scalar.activation(S_exp0[:, sb, :], ps[:],
                                     mybir.ActivationFunctionType.Exp,
                                     bias=mx[:, sb, 0:1],
                                     accum_out=Z[:, sb, 0:1])

            # -------- psi_2 graph-s precompute (both steps) --------
            y0ps = psmall.tile([128, STEPS, NB, R], F32, tag="small", name="y0ps")
            for k in range(STEPS):
                for nb_ in range(NB):
                    nc.tensor.matmul(y0ps[:, k, nb_, :],
                                     lhsT=rst_k[k][:, nb_ * 128:(nb_ + 1) * 128],
                                     rhs=w2n[:], start=True, stop=True,
                                     skip_group_check=True)
            y0packed = work.tile([128, STEPS, NB, R], F32R, tag="y0packed",
                                 name="y0packed")
            nc.vector.tensor_copy(y0packed[:], y0ps[:])
            A_packed = []
            for k in range(STEPS):
                osps = psmall.tile([R, N], F32, tag="small", name=f"osps{k}")
                nc.tensor.matmul(osps[:], lhsT=w2r[:], rhs=rst_k[k][:],
                                 start=True, stop=False, skip_group_check=True)
                add_aggT(osps, A_sr, y0packed[:, k])
                o_sT = work.tile([R, N], F32R, tag="o_sT", name=f"o_sT{k}")
                nc.scalar.activation(o_sT[:], osps[:],
                                     mybir.ActivationFunctionType.Relu, bias=b2c[:, :1])
                a4ps = pbig.tile([128, N], F32, tag="big", name=f"a4ps{k}")
                nc.tensor.matmul(a4ps[:], lhsT=wm1p4[:], rhs=o_sT[:],
                                 start=True, stop=True)
                ap_t = work.tile([128, 128], F32, tag="A_packed", name=f"A_packed{k}")
                a4v = a4ps[:].rearrange("p (m j) -> p j m", j=4)
                for j in range(4):
                    nc.vector.tensor_scalar(
                        out=ap_t[R * j:R * (j + 1), :],
                        in0=a4v[R * j:R * (j + 1), j, :],
                        scalar1=bm1c[:, 0:1], scalar2=None,
                        op0=mybir.AluOpType.add)
                A_packed.append(ap_t)

            # -------- consensus steps --------
            S0_sb = work.tile([128, NB, N], F32R, tag="S0_sb", name="S0_sb")
            S_exp1 = work.tile([128, NB, N], F32R, tag="S_exp1", name="S_exp1")
            SL_sb = work.tile([128, NB, N], F32, tag="SL_sb", name="SL_sb")
            rsp = work.tile([128, NB, R], F32R, tag="rsp", name="rsp")
            S0v = S0.rearrange("(a b) t -> a b t", b=128)
            SLv = SL.rearrange("(a b) t -> a b t", b=128)
            S_exp_k = [S_exp0, S_exp1]

            for k in range(STEPS):
                # r_t^T via direct accumulation over s-blocks
                rtT_ps = psmall.tile([R, N], F32, tag="small", name=f"rtTps{k}")
                for sb in range(NB):
                    nc.vector.reciprocal(rz[:, sb, k:k + 1], Z[:, sb, k:k + 1])
                    if k == 0:
                        nc.vector.tensor_scalar(
                            out=S0_sb[:, sb, :], in0=S_exp0[:, sb, :],
                            scalar1=rz[:, sb, 0:1], scalar2=None,
                            op0=mybir.AluOpType.mult)
                        nc.sync.dma_start(S0v[sb], S0_sb[:, sb, :])
                        nc.tensor.matmul(rtT_ps[:], lhsT=rsr[:, 0, sb, :],
                                         rhs=S0_sb[:, sb, :], start=(sb == 0),
                                         stop=(sb == NB - 1), skip_group_check=True)
                    else:
                        nc.vector.tensor_scalar(
                            out=rsp[:, sb, :], in0=rsr[:, 1, sb, :],
                            scalar1=rz[:, sb, 1:2], scalar2=None,
                            op0=mybir.AluOpType.mult)
                        nc.tensor.matmul(rtT_ps[:], lhsT=rsp[:, sb, :],
                                         rhs=S_exp1[:, sb, :], start=(sb == 0),
                                         stop=(sb == NB - 1), skip_group_check=True)
                rtT = work.tile([R, N], F32R, tag="rtT", name=f"rtT{k}")
                nc.scalar.copy(rtT[:], rtT_ps[:])

                # o_t GNN
                y1ps = psmall.tile([128, NB, R], F32, tag="small", name=f"y1ps{k}")
                for nb_ in range(NB):
                    nc.tensor.matmul(y1ps[:, nb_, :], lhsT=rtT[:, nb_ * 128:(nb_ + 1) * 128],
                                     rhs=w2n[:], start=True, stop=True,
                                     skip_group_check=True)
                y1_rows = work.tile([128, NB, R], F32R, tag="y1_rows", name=f"y1_rows{k}")
                nc.vector.tensor_copy(y1_rows[:], y1ps[:])
                otps = psmall.tile([R, N], F32, tag="small", name=f"otT{k}")
                nc.tensor.matmul(otps[:], lhsT=w2r[:], rhs=rtT[:], start=True, stop=False,
                                 skip_group_check=True)
                add_aggT(otps, A_tr, y1_rows)
                o_tT = work.tile([R, N], F32R, tag="o_tT", name=f"o_tT{k}")
                nc.scalar.activation(o_tT[:], otps[:], mybir.ActivationFunctionType.Relu,
                                     bias=b2c[:, :1])

                # -cT replicated 4x via tiled -Wm1
                c4ps = pbig.tile([128, N], F32, tag="big", name=f"c4ps{k}")
                nc.tensor.matmul(c4ps[:], lhsT=wm1n4[:], rhs=o_tT[:],
                                 start=True, stop=True)
                cT4neg = work.tile([128, N], FP16, tag="cT4neg", name=f"cT4neg{k}")
                nc.scalar.copy(cT4neg[:], c4ps[:])

                # -------- mlp loop (128 relu ops + 128 matmuls) --------
                mlp_ps = [pbig.tile([128, N], F32, tag="big", name=f"mlp{k}{b_}")
                          for b_ in range(NB)]
                for blk in range(NB):
                    for mi in range(32):
                        m = blk * 32 + mi
                        grp, v = mi // 8, mi % 8
                        r4 = r4p.tile([128, N], FP16, tag="r4")
                        eng = R4_ENGINE[v]
                        if eng == "dve":
                            nc.vector.tensor_scalar(
                                out=r4[:], in0=cT4neg[:],
                                scalar1=A_packed[k][:, m:m + 1], scalar2=0.0,
                                op0=mybir.AluOpType.add, op1=mybir.AluOpType.max)
                        elif eng == "pool":
                            nc.gpsimd.tensor_scalar(
                                out=r4[:], in0=cT4neg[:],
                                scalar1=A_packed[k][:, m:m + 1], scalar2=0.0,
                                op0=mybir.AluOpType.add, op1=mybir.AluOpType.max)
                        else:
                            nc.scalar.activation(r4[:], cT4neg[:],
                                                 mybir.ActivationFunctionType.Relu,
                                                 bias=A_packed[k][:, m:m + 1])
                        if grp < 2:
                            nc.tensor.matmul(mlp_ps[blk][32 * grp:32 * (grp + 1), :],
                                             lhsT=w2p[:, v, 0:32], rhs=r4[:],
                                             start=(v == 0), stop=(v == 7))
                        elif grp == 2:
                            nc.tensor.matmul(mlp_ps[blk][64:128, :],
                                             lhsT=w2p[:, v, 0:64], rhs=r4[:],
                                             start=(v == 0), stop=False)
                        else:
                            nc.tensor.matmul(mlp_ps[blk][64:128, :],
                                             lhsT=w2p[:, v, 64:128], rhs=r4[:],
                                             start=False, stop=(v == 7))
                    # block blk of mlp done: update S_hat, then next softmax piece
                    nc.vector.tensor_tensor(out=S_hat[:, blk, :], in0=S_hat[:, blk, :],
                                            in1=mlp_ps[blk][:], op=mybir.AluOpType.add)
                    if k == 0:
                        nc.vector.tensor_reduce(mx[:, blk, 1:2], S_hat[:, blk, :],
                                                axis=mybir.AxisListType.X,
                                                op=mybir.AluOpType.max, negate=True)
                        nc.scalar.activation(S_exp1[:, blk, :], S_hat[:, blk, :],
                                             mybir.ActivationFunctionType.Exp,
                                             bias=mx[:, blk, 1:2],
                                             accum_out=Z[:, blk, 1:2])
                    else:
                        nc.vector.tensor_reduce(mx[:, blk, 2:3], S_hat[:, blk, :],
                                                axis=mybir.AxisListType.X,
                                                op=mybir.AluOpType.max, negate=True)
                        nc.scalar.activation(SL_sb[:, blk, :], S_hat[:, blk, :],
                                             mybir.ActivationFunctionType.Exp,
                                             bias=mx[:, blk, 2:3],
                                             accum_out=Z[:, blk, 2:3])
                        nc.vector.reciprocal(rz[:, blk, 2:3], Z[:, blk, 2:3])
                        nc.vector.tensor_scalar(
                            out=SL_sb[:, blk, :], in0=SL_sb[:, blk, :],
                            scalar1=rz[:, blk, 2:3], scalar2=None,
                            op0=mybir.AluOpType.mult)
                        nc.sync.dma_start(SLv[blk], SL_sb[:, blk, :])

        if repeats == 1:
            body()
        else:
            assert repeats % unroll == 0
            with tc.For_i(0, repeats // unroll, 1):
                for u in range(unroll):
                    body(u)

    nc.compile()
    return nc


def prep_core_inputs(g, inp):
    n0 = g * N
    xs = np.asarray(inp["x_s"][n0:n0 + N], np.float32)
    xt = np.asarray(inp["x_t"][n0:n0 + N], np.float32)
    es = np.asarray(inp["edge_index_s"][:, g * E:(g + 1) * E]).astype(np.int64) - n0
    et = np.asarray(inp["edge_index_t"][:, g * E:(g + 1) * E]).astype(np.int64) - n0
    eas = np.asarray(inp["edge_attr_s"][g * E:(g + 1) * E, 0], np.float64)
    eat = np.asarray(inp["edge_attr_t"][g * E:(g + 1) * E, 0], np.float64)
    rs = np.asarray(inp["r_s_all"][:, g], np.float32)

    def a_build(edges, ea):
        at = np.bincount(edges[0] * N + edges[1], weights=ea, minlength=N * N)
        at = at.astype(np.float32).reshape(NB, 128, N).transpose(1, 0, 2)
        return np.ascontiguousarray(at)

    w2 = np.asarray(inp["Wm2"], np.float32)[:, 0]
    w2p = np.zeros((128, 8, 128), np.float32)
    for v in range(8):
        for j in range(4):
            w2p[R * j:R * (j + 1), v, 4 * v + j] = w2
            w2p[R * j:R * (j + 1), v, 96 + 4 * v + j] = w2
    wm1 = np.asarray(inp["Wm1"], np.float32)
    wm1p4 = np.concatenate([wm1] * 4, axis=1)
    return {
        "xsT": np.ascontiguousarray(xs.T),
        "xtT": np.ascontiguousarray(xt.T),
        "rs_rows": np.ascontiguousarray(rs.reshape(STEPS, NB, 128, R).transpose(2, 0, 1, 3)),
        "rsT": np.ascontiguousarray(rs.transpose(0, 2, 1)),
        "A_rows": np.stack([a_build(es, eas), a_build(et, eat)]),
        "W1r": np.asarray(inp["W1_root"], np.float32),
        "W1n": np.asarray(inp["W1_nbr"], np.float32),
        "b1": np.asarray(inp["b1"], np.float32).reshape(DH, 1),
        "W2r": np.asarray(inp["W2_root"], np.float32),
        "W2n": np.asarray(inp["W2_nbr"], np.float32),
        "b2": np.asarray(inp["b2"], np.float32).reshape(R, 1),
        "Wm1p4": wm1p4,
        "Wm1n4": -wm1p4,
        "bm1": np.asarray(inp["bm1"], np.float32).reshape(R, 1),
        "W2p": w2p.astype(np.float16),
    }


_NC_CACHE = {}


def _get_nc(repeats=1, unroll=1, r4_engine=None, r4bufs=12):
    key = (repeats, unroll, tuple(sorted(r4_engine.items())) if r4_engine else None, r4bufs)
    if key not in _NC_CACHE:
        _NC_CACHE[key] = build_kernel(repeats, unroll, r4_engine, r4bufs)
    return _NC_CACHE[key]


def kernel(**inputs):
    from concourse.bass_utils import run_bass_kernel_spmd
    nc = _get_nc(1)
    in_maps = [prep_core_inputs(g, inputs) for g in range(B)]
    res = run_bass_kernel_spmd(nc, in_maps, core_ids=list(range(N_CORES)))
    S0 = np.stack([res.results[g]["S0"] for g in range(B)])
    SL = np.stack([res.results[g]["SL"] for g in range(B)])
    return S0, SL
